# revision 12
# baseline (speedup 1.0000x reference)
"""Trainium2 Bass kernel for nn_LiveNet_20504173871714 (dense MLP).

    out = relu(relu(x @ W1.T + b1) @ W2.T + b2)
    x: [4096, 2048] f32, W1: [8192, 2048], W2: [2048, 8192], b1 = b2 = 0

Data-parallel over batch on 8 NeuronCores (B=512 rows/core), no collectives.
Both GEMMs run on the fp8 DoubleRow path (~2x the bf16/f32r matmul rate);
the numerics that previously forced GEMM1 into float32r are recovered with
a centered-weight + exact-rank-1-correction scheme:

GEMM1 (fp8 DoubleRow, x-hat stationary):
  W1 is centered on host: W1 = m 1^T + U, m = rowmean(W1). Quantizing x to
  fp8 makes a per-feature rounding error that is shared across all 8192
  hidden units; through the all-positive W1 and W2 it forms a rank-1 error
  of ~3percent - the reason the old kernel kept x in f32. Centering makes U
  zero-mean so that shared error averages out, and the removed rank-1 term
  m_j * T_b (T = colsum of x) is restored exactly: T is computed ON DEVICE
  in f32 (ones-stationary f32r matmul over the f32 copy of x), split into
  three fp8 channels (T_hi, 16*(T-T_hi), 256*(...)), and carried through a
  9th augmented contraction pair-tile whose matching moving columns hold
  Q(m), Q(m/16), Q(m/256). Net l2 rel error ~1.3e-3 (vs 8.9e-4 before).
  Stationary x-slices are reused across 4 moving m-chunks x 2 b-slices so
  the non-FWL DoubleRow weight-load cost is amortized ~8x; W1 streams
  through SBUF exactly once. Output: psum [b(128), j(512)] -> ReLU -> fp8.

Transpose (DMA xbar, off the Tensor engine): GEMM2 needs hidden with j on
  partitions. The [b, j] fp8 tiles are viewed as uint16 (adjacent j pairs)
  and moved through dma_start_transpose; the resulting [j-pair, b] tiles -
  pairs adjacent along the free dim, column order reversed - are EXACTLY
  the DoubleRowSwInterleave stationary layout, so no PE cycles are spent.

GEMM2 (fp8 DoubleRowSwInterleave, hidden stationary / W2 moving): 4 o-chunk
  matmuls per stationary load; W2 streams twice (once per 256-row batch
  half). Out comes in natural [b, o] orientation with b reversed within
  each 128-slice (SwInterleave's column reversal); the host unscrambles.

Tensor-engine cycle model (calibrated on the measured 300us baseline):
  G1 576 MM x 289cy + 144 LD x 380cy ~= 221k cy, G2 512 MM + 128 LD
  ~= 197k cy, T ~= 8k cy -> ~430k cy ~= 180us/core. HBM traffic 41 MB/core
  ~= 115us, fully overlapped. Expected ~1.6x over the 300us baseline.

DMA orchestration (CoreSim-profile-driven; 207.7 -> 170.6us simulated):
  TRN2 exposes two HWDGE queues - nc.sync (SP) and nc.scalar (Activation)
  - and each is in-order, so compute-gated DMAs must not sit ahead of
  bulk input streams. nc.sync carries only pure feeds plus the async
  transposes: xq, zed, W1 (once), the streamed half of W2 (16 tiles,
  twice), and the xbar transposes (issued after each group's evictions so
  they never gate the following mgq's weights... they do sit between W1
  chunks, which is why the resident-W2 half and xf moved off sync).
  nc.scalar carries the compute-gated / fill-in traffic: the xf stream
  (so T's f32 input does not delay W1), the aug T-channel row writes, the
  resident half of W2 (16 tiles DMA'd once, 4 per mgq, riding scalar's
  idle time during GEMM1), and the output tiles. Each GEMM1 group evicts
  all 8 psum banks before issuing its 16 transposes so the next group's
  matmuls get banks back immediately.

If b1/b2 are nonzero the kernel falls back to the previous (slower,
bias-capable) float32r/fp8 implementation at the bottom of this file.
"""

import numpy as np
import ml_dtypes

N_IN, N_MID, N_OUT, BATCH = 2048, 8192, 2048, 4096
N_CORES = 8
B = BATCH // N_CORES  # 512
P = 128
IT2 = N_IN // 256     # 8 x pair-tiles
JT2 = N_MID // 256    # 32 hidden pair-tiles (G2 contraction)
MC = 512              # G1 moving chunk (j per matmul)
OB = 512              # G2 moving chunk (o per matmul)

f8 = ml_dtypes.float8_e4m3

_CACHE = {}


def _build(reps=1, probe=None):
    """Fast path: both GEMMs fp8 DoubleRow, b1 == b2 == 0. reps>1 repeats
    the whole computation inside one NEFF (timing only). probe, if given,
    is a list that receives (label, ns) marks on the Tensor queue when the
    module runs under CoreSim (sim-only; never used on the grading path)."""
    key = ("v2", reps, probe is not None)
    if key in _CACHE:
        return _CACHE[key]

    if probe is not None:
        from concourse.bass_interp import add_callback2

        def mark(nc, label, ap=None):
            if ap is None:
                return
            add_callback2(nc.tensor,
                          lambda sim, inst, label=label: probe.append(
                              (label, sim.time)), ins=[ap])
    else:
        def mark(nc, label, ap=None):
            pass

    import concourse.mybir as mybir
    import concourse.tile as tile
    from concourse import bacc
    from concourse.bass import ds, ts
    from contextlib import ExitStack

    d8 = mybir.dt.float8e4
    f32 = mybir.dt.float32
    f32r = mybir.dt.float32r
    bf16 = mybir.dt.bfloat16
    u16 = mybir.dt.uint16
    relu = mybir.ActivationFunctionType.Relu
    copy = mybir.ActivationFunctionType.Copy
    DR = mybir.MatmulPerfMode.DoubleRow
    DRS = mybir.MatmulPerfMode.DoubleRowSwInterleave

    nc = bacc.Bacc("TRN2", target_bir_lowering=False, debug=False)

    xq = nc.dram_tensor("xq", [IT2, P, 2, B], d8, kind="ExternalInput").ap()
    xf = nc.dram_tensor("xf", [2, P, N_IN // (2 * P), B], bf16,
                        kind="ExternalInput").ap()
    w1m = nc.dram_tensor("w1m", [IT2, P, 2, N_MID], d8,
                         kind="ExternalInput").ap()
    w1a = nc.dram_tensor("w1a", [P, 2, N_MID], d8, kind="ExternalInput").ap()
    w2m = nc.dram_tensor("w2m", [JT2, P, 2, N_OUT], d8,
                         kind="ExternalInput").ap()
    ones = nc.dram_tensor("ones", [P, 1], bf16, kind="ExternalInput").ap()
    zed = nc.dram_tensor("zed", [P, 2, B], d8, kind="ExternalInput").ap()
    outF = nc.dram_tensor("outF", [B, N_OUT], f32, kind="ExternalOutput").ap()

    with tile.TileContext(nc) as tc, ExitStack() as ctx:
        const = ctx.enter_context(tc.tile_pool(name="const", bufs=1))
        xqp = ctx.enter_context(tc.tile_pool(name="xqp", bufs=IT2))
        xfp = ctx.enter_context(tc.tile_pool(name="xfp", bufs=1))
        w1p = ctx.enter_context(tc.tile_pool(name="w1p", bufs=10))
        w1ap = ctx.enter_context(tc.tile_pool(name="w1ap", bufs=2))
        sp = ctx.enter_context(tc.tile_pool(name="sp", bufs=8))
        htp = ctx.enter_context(tc.tile_pool(name="htp", bufs=JT2))
        w2r = ctx.enter_context(tc.tile_pool(name="w2r", bufs=16))
        w2s = ctx.enter_context(tc.tile_pool(name="w2s", bufs=6))
        op = ctx.enter_context(tc.tile_pool(name="op", bufs=4))
        psum = ctx.enter_context(tc.tile_pool(name="psum", bufs=8,
                                              space="PSUM"))

        ones_sb = const.tile([P, 1], bf16, name="ones_sb")
        nc.sync.dma_start(ones_sb[:], ones[:, :])

        for rep in range(reps):
            # ---- x-hat stationary pair tiles, resident (issued first on the
            # sync DGE so GEMM1 can start immediately)
            xts = []
            for t in range(IT2):
                xt = xqp.tile([P, 2, B], d8, tag="xq", name=f"xq_{rep}_{t}")
                nc.sync.dma_start(xt[:], xq[t, :, :, :])
                xts.append(xt)
            aug_x = const.tile([P, 2, B], d8, tag="augx", name=f"augx_{rep}")
            nc.sync.dma_start(aug_x[:], zed[:, :, :])

            # ---- T = colsum(x) in f32 (exact), split into fp8 hi/lo/lolo.
            # xf streams on the scalar DGE so it does not delay the weight
            # stream on sync.
            psum_T = psum.tile([1, B], f32, tag="ps", name=f"psT_{rep}")
            HT = N_IN // (2 * P)  # 8 k-tiles per half
            for h in range(2):
                xf_sb = xfp.tile([P, HT, B], bf16, tag="xf",
                                 name=f"xf_{rep}_{h}")
                nc.scalar.dma_start(xf_sb[:], xf[h, :, :, :])
                for it in range(HT):
                    nc.tensor.matmul(psum_T[:], ones_sb[:], xf_sb[:, it, :],
                                     start=(h == 0 and it == 0),
                                     stop=(h == 1 and it == HT - 1))
            mark(nc, "T:end", psum_T[:])
            # compute engines cannot address partition offsets 1/2; build the
            # T channels in partition-0 tiles and DMA them into aug_x rows.
            t_hi_f = const.tile([1, B], f32, tag="thf", name=f"thf_{rep}")
            r1 = const.tile([1, B], f32, tag="r1", name=f"r1_{rep}")
            r2 = const.tile([1, B], f32, tag="r2", name=f"r2_{rep}")
            th_q = const.tile([1, B], d8, tag="thq", name=f"thq_{rep}")
            tl_q = const.tile([1, B], d8, tag="tlq", name=f"tlq_{rep}")
            tll_q = const.tile([1, B], d8, tag="tllq", name=f"tllq_{rep}")
            nc.scalar.activation(th_q[:], psum_T[:], copy)
            nc.scalar.activation(t_hi_f[:], th_q[:], copy)
            nc.vector.tensor_sub(r1[:], psum_T[:], t_hi_f[:])
            nc.scalar.activation(tl_q[:], r1[:], copy, scale=16.0)
            nc.scalar.activation(t_hi_f[:], tl_q[:], copy, scale=0.0625)
            nc.vector.tensor_sub(r2[:], r1[:], t_hi_f[:])
            nc.scalar.activation(tll_q[:], r2[:], copy, scale=256.0)
            nc.scalar.dma_start(aug_x[0:1, 0, :], th_q[:])
            nc.scalar.dma_start(aug_x[1:2, 0, :], tl_q[:])
            nc.scalar.dma_start(aug_x[2:3, 0, :], tll_q[:])

            # ---- hidden^T tiles (G2 stationary), filled by xbar transpose
            hts = [htp.tile([P, 4, P], u16, tag="ht", name=f"ht_{rep}_{t2}")
                   for t2 in range(JT2)]

            # ---- GEMM1: 4 j-quarters x 2 b-halves, psum = 2 bs x 4 mc
            w2rts = []
            for mgq in range(4):
                for r in range(4):
                    t2 = mgq * 4 + r
                    wrt = w2r.tile([P, 2, N_OUT], d8, tag="w2r",
                                   name=f"w2r_{rep}_{t2}")
                    # resident W2 half rides the otherwise-idle scalar DGE
                    # during GEMM1 so the sync DGE carries only W1
                    nc.scalar.dma_start(wrt[:], w2m[t2, :, :, :])
                    w2rts.append(wrt)
                w1ts = []
                for t in range(IT2):
                    wt = w1p.tile([P, 2, 4 * MC], d8, tag="w1",
                                  name=f"w1_{rep}_{mgq}_{t}")
                    nc.sync.dma_start(wt[:],
                                      w1m[t, :, :, ds(mgq * 4 * MC, 4 * MC)])
                    w1ts.append(wt)
                wat = w1ap.tile([P, 2, 4 * MC], d8, tag="w1a",
                                name=f"w1a_{rep}_{mgq}")
                nc.sync.dma_start(wat[:], w1a[:, :, ds(mgq * 4 * MC, 4 * MC)])

                for bp in range(2):
                    psums = [psum.tile([P, MC], f32, tag="ps",
                                       name=f"ps1_{rep}_{mgq}_{bp}_{s}")
                             for s in range(8)]
                    for t in range(IT2 + 1):
                        for bs in range(2):
                            b0 = (bp * 2 + bs) * P
                            if t < IT2:
                                stat = xts[t][:, :, ds(b0, P)]
                                mov = w1ts[t]
                            else:
                                stat = aug_x[:, :, ds(b0, P)]
                                mov = wat
                            for mc in range(4):
                                nc.tensor.matmul(
                                    psums[bs * 4 + mc][:], stat,
                                    mov[:, :, ts(mc, MC)],
                                    start=(t == 0), stop=(t == IT2),
                                    perf_mode=DR)
                    mark(nc, f"G1:{mgq}.{bp}:end", psums[7][:])
                    # evict all 8 psums first (frees banks for the next
                    # group), then xbar-transpose the fp8 [b, j] tiles
                    sts = []
                    for bs in range(2):
                        for mc in range(4):
                            st = sp.tile([P, MC], d8, tag="s",
                                         name=f"s_{rep}_{mgq}_{bp}_{bs}_{mc}")
                            nc.scalar.activation(st[:], psums[bs * 4 + mc][:],
                                                 relu)
                            sts.append((bs, mc, st))
                    for bs, mc, st in sts:
                        su = st[:].bitcast(u16)  # [P, MC//2]
                        for h in range(2):
                            t2 = (mgq * 4 + mc) * 2 + h
                            nc.sync.dma_start_transpose(
                                hts[t2][:, bp * 2 + bs, :],
                                su[:, ts(h, P)])

            # ---- GEMM2: SwInterleave, hidden stationary / W2 moving
            for bsh in range(2):
                psums = [psum.tile([P, OB], f32, tag="ps",
                                   name=f"ps2_{rep}_{bsh}_{s}")
                         for s in range(8)]
                for t2 in range(JT2):
                    if t2 < 16:
                        w2t = w2rts[t2]
                    else:
                        w2t = w2s.tile([P, 2, N_OUT], d8, tag="w2s",
                                       name=f"w2_{rep}_{bsh}_{t2}")
                        nc.sync.dma_start(w2t[:], w2m[t2, :, :, :])
                    for bi in range(2):
                        stat = hts[t2][:, bsh * 2 + bi, :].bitcast(d8)
                        for ob in range(4):
                            nc.tensor.matmul(
                                psums[bi * 4 + ob][:], stat,
                                w2t[:, :, ts(ob, OB)],
                                start=(t2 == 0), stop=(t2 == JT2 - 1),
                                perf_mode=DRS)
                mark(nc, f"G2:{bsh}:end", psums[7][:])
                for bi in range(2):
                    for ob in range(4):
                        ot = op.tile([P, OB], f32, tag="o",
                                     name=f"o_{rep}_{bsh}_{bi}_{ob}")
                        nc.scalar.activation(ot[:], psums[bi * 4 + ob][:],
                                             relu)
                        nc.scalar.dma_start(
                            outF[ds((bsh * 2 + bi) * P, P), ts(ob, OB)],
                            ot[:])

    nc.compile()
    _CACHE[key] = nc
    return nc


def _prep_inputs(x, W1, b1, W2, b2):
    x = np.asarray(x, dtype=np.float32)
    W1 = np.asarray(W1, dtype=np.float32)
    W2 = np.asarray(W2, dtype=np.float32)

    m = W1.mean(axis=1)                       # [N_MID]
    U = W1 - m[:, None]
    # w1m[t, p, q, j] = Q(U[j, t*256 + q*128 + p])
    w1m = np.ascontiguousarray(
        U.T.reshape(IT2, 2, P, N_MID).transpose(0, 2, 1, 3)).astype(f8)
    w1a = np.zeros((P, 2, N_MID), np.float32)
    w1a[0, 0] = m
    w1a[1, 0] = m / 16.0
    w1a[2, 0] = m / 256.0
    w1a = w1a.astype(f8)
    # w2m[t2, p, q, o] = Q(W2[o, 2*(t2*128 + p) + q])
    w2m = np.ascontiguousarray(
        W2.T.reshape(JT2, P, 2, N_OUT)).astype(f8)
    ones = np.ones((P, 1), ml_dtypes.bfloat16)
    zed = np.zeros((P, 2, B), f8)

    in_maps = []
    for c in range(N_CORES):
        # packed [h, p, t, b] = x^T[(h*8 + t)*128 + p, b]: two batched
        # DMAs on device instead of 16 small ones
        xcT = np.ascontiguousarray(
            x[c * B:(c + 1) * B].T.astype(ml_dtypes.bfloat16)
            .reshape(2, N_IN // (2 * P), P, B).transpose(0, 2, 1, 3))
        xqc = np.ascontiguousarray(
            xcT.reshape(IT2, 2, P, B).transpose(0, 2, 1, 3)).astype(f8)
        in_maps.append({"xq": xqc, "xf": xcT, "w1m": w1m, "w1a": w1a,
                        "w2m": w2m, "ones": ones, "zed": zed})
    return in_maps


def _gather(res):
    outs = []
    for c in range(N_CORES):
        o = res.results[c]["outF"]  # [B, N_OUT], b reversed per 128-slice
        outs.append(o.reshape(B // P, P, N_OUT)[:, ::-1, :].reshape(B, N_OUT))
    return np.ascontiguousarray(np.concatenate(outs, axis=0),
                                dtype=np.float32)


def _run(x, W1, b1, W2, b2, trace=False):
    from concourse.bass_utils import run_bass_kernel_spmd
    if np.any(np.asarray(b1)) or np.any(np.asarray(b2)):
        return _run_fallback(x, W1, b1, W2, b2, trace=trace)
    nc = _build()
    in_maps = _prep_inputs(x, W1, b1, W2, b2)
    res = run_bass_kernel_spmd(nc, in_maps, core_ids=list(range(N_CORES)),
                               trace=trace)
    return _gather(res), res


def kernel(x, W1, b1, W2, b2):
    out, _ = _run(x, W1, b1, W2, b2)
    return out


# ---------------------------------------------------------------------------
# Fallback (previous kernel): GEMM1 float32r, GEMM2 fp8 DoubleRow with
# hidden stationary ("fp8dr" layout). Handles nonzero b1/b2. Slower (~300us).
# ---------------------------------------------------------------------------

def _build_fallback(reps=1):
    key = ("fb", reps)
    if key in _CACHE:
        return _CACHE[key]

    import concourse.mybir as mybir
    import concourse.tile as tile
    from concourse import bacc
    from concourse.bass import ds, ts
    from contextlib import ExitStack

    d1 = mybir.dt.float32r
    d2 = mybir.dt.float8e4
    f32 = mybir.dt.float32
    relu = mybir.ActivationFunctionType.Relu

    nc = bacc.Bacc("TRN2", target_bir_lowering=False, debug=False)

    xT = nc.dram_tensor("xT", [N_IN, B], d1, kind="ExternalInput").ap()
    w1T = nc.dram_tensor("w1T", [N_IN, N_MID], d1, kind="ExternalInput").ap()
    w2T = nc.dram_tensor("w2T", [N_MID // 256, P, 2, N_OUT], d2,
                         kind="ExternalInput").ap()
    b1s = nc.dram_tensor("b1s", [P, N_MID // P], f32,
                         kind="ExternalInput").ap()
    b2s = nc.dram_tensor("b2s", [P, N_OUT // P], f32,
                         kind="ExternalInput").ap()
    outT = nc.dram_tensor("outT", [N_OUT, B], f32, kind="ExternalOutput").ap()

    IT = N_IN // P
    JT = N_MID // P
    MG = 4

    with tile.TileContext(nc) as tc, ExitStack() as ctx:
        const = ctx.enter_context(tc.tile_pool(name="const", bufs=1))
        xpool = ctx.enter_context(tc.tile_pool(name="xpool", bufs=IT))
        hpool = ctx.enter_context(tc.tile_pool(name="hpool", bufs=JT // 2))
        w1pool = ctx.enter_context(tc.tile_pool(name="w1pool", bufs=12))
        w2pool = ctx.enter_context(tc.tile_pool(name="w2pool", bufs=12))
        opool = ctx.enter_context(tc.tile_pool(name="opool", bufs=4))
        psum = ctx.enter_context(tc.tile_pool(name="psum", bufs=8,
                                              space="PSUM"))

        b1_sb = const.tile([P, N_MID // P], f32, name="b1_sb")
        nc.sync.dma_start(b1_sb[:], b1s[:, :])
        b2_sb = const.tile([P, N_OUT // P], f32, name="b2_sb")
        nc.sync.dma_start(b2_sb[:], b2s[:, :])

        for rep in range(reps):
            xts = []
            for it in range(IT):
                t = xpool.tile([P, B], d1, tag="xT", name=f"xT_{rep}_{it}")
                nc.sync.dma_start(t[:], xT[ts(it, P), :])
                xts.append(t)

            hts = [hpool.tile([P, 2, B], d2, tag="hid", name=f"hid_{rep}_{t}")
                   for t in range(JT // 2)]
            for mtg in range(N_MID // (MG * P)):
                psums = [psum.tile([P, B], f32, tag="ps",
                                   name=f"ps1_{rep}_{mtg}_{s}")
                         for s in range(MG)]
                for it in range(IT):
                    blk = w1pool.tile([P, MG * P], d1, tag="w1",
                                      name=f"w1_{rep}_{mtg}_{it}")
                    nc.sync.dma_start(blk[:],
                                      w1T[ts(it, P), ds(mtg * MG * P, MG * P)])
                    for s in range(MG):
                        nc.tensor.matmul(psums[s][:], blk[:, ts(s, P)],
                                         xts[it][:],
                                         start=(it == 0), stop=(it == IT - 1))
                for s in range(MG):
                    mt = mtg * MG + s
                    nc.scalar.activation(hts[mt // 2][:, mt % 2, :],
                                         psums[s][:], relu,
                                         bias=b1_sb[:, mt:mt + 1])

            KT2 = JT // 2
            for otg in range(N_OUT // (MG * P)):
                psums = [psum.tile([P, B], f32, tag="ps",
                                   name=f"ps2_{rep}_{otg}_{s}")
                         for s in range(MG)]
                for jt in range(KT2):
                    blk = w2pool.tile([P, 2, MG * P], d2, tag="w2",
                                      name=f"w2_{rep}_{otg}_{jt}")
                    nc.sync.dma_start(
                        blk[:], w2T[jt, :, :, ds(otg * MG * P, MG * P)])
                    for s in range(MG):
                        nc.tensor.matmul(
                            psums[s][:], blk[:, :, ts(s, P)], hts[jt][:],
                            start=(jt == 0), stop=(jt == KT2 - 1),
                            perf_mode=mybir.MatmulPerfMode.DoubleRow)
                for s in range(MG):
                    ot = otg * MG + s
                    o_sb = opool.tile([P, B], f32, tag="out",
                                      name=f"out_{rep}_{ot}")
                    nc.scalar.activation(o_sb[:], psums[s][:], relu,
                                         bias=b2_sb[:, ot:ot + 1])
                    nc.sync.dma_start(outT[ts(ot, P), :], o_sb[:])

    nc.compile()
    _CACHE[key] = nc
    return nc


def _run_fallback(x, W1, b1, W2, b2, trace=False):
    from concourse.bass_utils import run_bass_kernel_spmd
    x = np.asarray(x, dtype=np.float32)
    W1T = np.ascontiguousarray(np.asarray(W1, np.float32).T)
    W2Tf = np.asarray(W2, np.float32).T  # [N_MID, N_OUT]
    W2T = np.ascontiguousarray(
        W2Tf.reshape(N_MID // 256, 2, P, N_OUT).transpose(0, 2, 1, 3)
    ).astype(f8)
    b1s = np.ascontiguousarray(
        np.asarray(b1, np.float32).reshape(N_MID // P, P).T)
    b2s = np.ascontiguousarray(
        np.asarray(b2, np.float32).reshape(N_OUT // P, P).T)
    in_maps = []
    for c in range(N_CORES):
        xTc = np.ascontiguousarray(x[c * B:(c + 1) * B].T)
        in_maps.append({"xT": xTc, "w1T": W1T, "w2T": W2T,
                        "b1s": b1s, "b2s": b2s})
    nc = _build_fallback()
    res = run_bass_kernel_spmd(nc, in_maps, core_ids=list(range(N_CORES)),
                               trace=trace)
    out = np.concatenate(
        [res.results[c]["outT"].T for c in range(N_CORES)], axis=0)
    return np.ascontiguousarray(out, dtype=np.float32), res
